# revision 37
# baseline (speedup 1.0000x reference)
"""Multi-head attention (B=4, S=2048, E=1024, H=16) on 8 TRN2 NeuronCores.

Sharding: core c -> (batch b = c//2, head-half hh = c%2  => 8 heads = 512 features).

v4 design (~404us HW, from 485us v2; trace-driven):
 - v2 was exp-bound in P2: 22 exp ACTIVATEs/group @ ~0.72us = 15.8us > PE
   ~14us/group, causing PE stalls, HAM cold-clock oscillation (~127us at
   1.2GHz) and 343us of P2.
 - v3 widens exp to head-PAIRS: both tile-position score halves land in one
   [128,1024] 2-bank PSUM tile; exp is ONE wide op per pair: ACT
   (1024+352)/1.2 = 1.15us (573ns/tile, was 720) for 11-12 pairs/group, DVE
   2-op custom chain over [128,1024] = 2.4us (1.2us/tile, was 1.36) for 4-5
   pairs/group -> ACT ~13.6us, DVE ~14.0us, PE ~13.9us per group: balanced.
 - KEY DECOUPLING: ctx matmuls are emitted LAG=10 kt-slots behind
   scores/exp (pt output buffered in a deep SBUF pool), so by the time a
   ctx pair issues its exp finished long ago -- the PE stopped waiting on
   in-flight ACT/DVE ops, HAM stays warm (cold 127us -> ~27us), and the
   LDWEIGHTS bubbles vanished (P2 median MM cadence 217ns ~= ideal 216).
 - ctx accumulates into one [65,1024] pair tile; softmax denominators: one
   full-tile DVE reciprocal from PSUM base-0 (DVE PSUM reads at partition
   base 64 return garbage -- probed; free-dim-bound so the 65-row recip
   costs the same as [1,1024]; rows 0-63 are unused junk), a tiny
   sbuf->sbuf DMA moves row 64 to partition 0 (idle queue), one wide
   gpsimd partition_broadcast, 2 DVE muls.  This removed the former
   1.15us/group ScalarE rowsum copy that was queue-limiting ACT.
 - A dummy ACTIVATE preloads the exp table set (~2.7us) during P1; first
   input DMAs split so the first V matmul starts ~5us earlier; output
   stored bf16 (halves store DMA; host upcasts and adds the two
   head-half partials).
 - V projection computed TRANSPOSED (x-tile stationary) so V lands directly
   in ctx-stationary layout [keys, head, dk]; V bias folded into host-side
   bo' = bo + Wo @ bv (softmax normalization makes it rowsum-invariant).
"""

import sys

sys.path.insert(0, "/opt/trn_rl_repo")

import numpy as np

B, S, E, H = 4, 2048, 1024, 16
DK = E // H  # 64
NCORES = 8
F = 512  # features per core (head-half)
SCALE = 1.0 / 8.0  # 1/sqrt(DK)

# ---------------------------------------------------------------- helpers

_EXP_OPS = None


def _register_exp_ops():
    """Two custom DVE ops for exp(x/8) on raw scores |x| <= ~28:
    EXPA_ANT: q = (((c3*x + c2)*x + c1)*x + 1)^4  ~= exp(x/128)
    SQ4_ANT:  out = in^16  (4 squarings)  => exp(x/8).
    """
    global _EXP_OPS
    if _EXP_OPS is not None:
        return _EXP_OPS
    import concourse.dve_ops as dve_ops
    from concourse.dve_ops import DveOp, DveOpSpec, get_dve_sub_opcode
    from concourse.dve_spec import Spec, Src0, C0, C1, C2, One, sq, lower

    existing = {op.name: op for op in dve_ops.OPS}
    if "EXPA_ANT" in existing and "SQ4_ANT" in existing:
        _EXP_OPS = (existing["EXPA_ANT"], existing["SQ4_ANT"])
        return _EXP_OPS

    def _ref_a(in0, in1, c0, c1, c2):
        x = in0.astype(np.float32)
        q = ((x * np.float32(c2) + np.float32(c1)) * x + np.float32(c0)) * x + np.float32(1.0)
        q = q * q
        return q * q

    def _ref_sq4(in0, in1, c0, c1, c2):
        x = in0.astype(np.float32)
        for _ in range(4):
            x = x * x
        return x

    opa = DveOp(
        "EXPA_ANT",
        Spec(body=sq(sq(((Src0 * C2 + C1) * Src0 + C0) * Src0 + One)), reference=_ref_a),
        subdim=False,
        uops_sha={},
    )
    opb = DveOp(
        "SQ4_ANT",
        Spec(body=sq(sq(sq(sq(Src0)))), reference=_ref_sq4),
        subdim=False,
        uops_sha={},
    )
    for op in (opa, opb):
        dve_ops.OPS.append(op)
        dve_ops._SUB_OPCODE_FOR_NAME[op.name] = (
            max(dve_ops._SUB_OPCODE_FOR_NAME.values()) + 1
        )
        dve_ops.CUSTOM_DVE_SPECS[op.name] = op.spec
        for ver in ("v3", "v4"):
            try:
                spec_c = DveOpSpec(
                    name=op.name,
                    opcode=get_dve_sub_opcode(op.name),
                    uops=lower(op.spec, ver=ver),
                    rd1_en=False,
                )
                op.uops_sha[ver] = spec_c.sha(ver)
            except Exception:
                pass
    _EXP_OPS = (opa, opb)
    return _EXP_OPS


EXPA_CONSTS = {
    "s0": 1.0 / 512.0,
    "s1": 1.0 / (2.0 * 512.0**2),
    "imm2": 1.0 / (6.0 * 512.0**3),
}

# per-group kts whose exp pair goes to the DVE (2-op wide chain); the rest
# go to ScalarE as one wide ACTIVATE.  Alternating 5/4 DVE pairs balances
# ACT (~1147ns/pair) and DVE (~2384ns/pair + ~2556ns recip+muls) at
# ~13.2us/group each.
_DVE_KTS_A = frozenset({1, 4, 7, 10, 13})
_DVE_KTS_B = frozenset({1, 4, 7, 10})

_BUILT = None  # cached compiled Bass program


def _build_program():
    global _BUILT
    if _BUILT is not None:
        return _BUILT

    import concourse.bass as bass
    import concourse.mybir as mybir
    from concourse import bacc
    from concourse.tile import TileContext

    EXPA, SQ4 = _register_exp_ops()

    F32 = mybir.dt.float32
    BF16 = mybir.dt.bfloat16
    AF = mybir.ActivationFunctionType

    nc = bacc.Bacc("TRN2", target_bir_lowering=False, debug=False, num_devices=NCORES)

    xq = nc.dram_tensor("xq", [E, S], BF16, kind="ExternalInput")
    xk = nc.dram_tensor("xk", [E, S], BF16, kind="ExternalInput")
    xv = nc.dram_tensor("xv", [E, S], BF16, kind="ExternalInput")
    wq = nc.dram_tensor("wq", [E, F], BF16, kind="ExternalInput")
    wk = nc.dram_tensor("wk", [E, F], BF16, kind="ExternalInput")
    wv = nc.dram_tensor("wv", [E, F], BF16, kind="ExternalInput")
    wo = nc.dram_tensor("wo", [F, E], BF16, kind="ExternalInput")
    bq = nc.dram_tensor("bq", [F], F32, kind="ExternalInput")
    bk = nc.dram_tensor("bk", [F], F32, kind="ExternalInput")
    out_d = nc.dram_tensor("out", [E, S], BF16, kind="ExternalOutput")

    with TileContext(nc) as tc:
        with (
            tc.tile_pool(name="persist", bufs=1) as persist,
            tc.tile_pool(name="xp", bufs=2) as xp,
            tc.tile_pool(name="ptp", bufs=4) as ptp,
            tc.tile_pool(name="smp", bufs=2) as smp,
            tc.tile_pool(name="ost", bufs=4) as ostp,
        ):
            QT = persist.tile([128, 4, S], BF16)
            KT = persist.tile([128, 4, S], BF16)
            Vn = persist.tile([128, 16, 8, 65], BF16)
            CX = persist.tile([128, 4, S], BF16)

            # ---------------- P0: ACT exp-table preload ----------
            junk = persist.tile([1, 8], BF16)
            nc.vector.memset(junk, 0.0)
            jexp = persist.tile([1, 8], F32)
            nc.scalar.activation(out=jexp, in_=junk, func=AF.Exp, scale=SCALE)

            # ---------------- P1: projections ----------------
            with (
                tc.tile_pool(name="wp1", bufs=1) as wp1,
                tc.tile_pool(name="mm1", bufs=3, space="PSUM") as mm1,
            ):
                # first DMAs on the queue: what the first matmul needs
                xv_r = xv.rearrange("(ec p) s -> p ec s", p=128)
                xch_next = xp.tile([128, 8, 512], BF16, tag="x", name="xch0")
                wv_sb = wp1.tile([128, 8, F], BF16, tag="wv")
                wv_r = wv.rearrange("(ec p) f -> p ec f", p=128)
                # split the first loads so the first matmul (needs ec=0 only)
                # starts as early as possible
                nc.sync.dma_start(out=xch_next[:, 0:2, :], in_=xv_r[:, 0:2, 0:512])
                nc.sync.dma_start(out=wv_sb[:, 0:2, :], in_=wv_r[:, 0:2, :])
                nc.sync.dma_start(out=xch_next[:, 2:8, :], in_=xv_r[:, 2:8, 0:512])
                nc.sync.dma_start(out=wv_sb[:, 2:8, :], in_=wv_r[:, 2:8, :])
                biases = persist.tile([128, 2, 4], F32)
                for ti, bt in enumerate((bq, bk)):
                    nc.sync.dma_start(
                        out=biases[:, ti, :],
                        in_=bt.rearrange("(ft p) -> p ft", p=128),
                    )
                # ones column for the rowsum trick (V stationary col 64)
                onec = persist.tile([128, 16, 8, 1], F32)
                nc.vector.memset(onec, 1.0)
                nc.vector.tensor_copy(out=Vn[:, :, :, 64:65], in_=onec)

                wq_sb = wp1.tile([128, 8, F], BF16, tag="wq")
                wk_sb = wp1.tile([128, 8, F], BF16, tag="wk")
                qk_xr = (
                    xq.rearrange("(ec p) s -> p ec s", p=128),
                    xk.rearrange("(ec p) s -> p ec s", p=128),
                )
                qk_stream = [(0, wq_sb, QT)] * 4 + [(1, wk_sb, KT)] * 4
                qk_next = None

                # V first: produced transposed ([s, f] = ctx-stationary layout)
                for sc in range(4):
                    xch = xch_next
                    if sc < 3:
                        ssl_n = slice((sc + 1) * 512, (sc + 2) * 512)
                        xch_next = xp.tile(
                            [128, 8, 512], BF16, tag="x", name=f"xch{sc+1}"
                        )
                        nc.sync.dma_start(out=xch_next, in_=xv_r[:, :, ssl_n])
                    for st in range(4):
                        stsl = slice(st * 128, (st + 1) * 128)
                        p = mm1.tile([128, 512], F32, tag="mm")
                        for ec in range(8):
                            nc.tensor.matmul(
                                p,
                                xch[:, ec, stsl],
                                wv_sb[:, ec, :],
                                start=(ec == 0),
                                stop=(ec == 7),
                            )
                        kti = sc * 4 + st
                        nc.vector.tensor_copy(
                            out=Vn[:, kti, :, 0:64],
                            in_=p.rearrange("p (h d) -> p h d", h=8),
                        )
                    if sc == 0:
                        # issue Q/K weight loads while V computes
                        nc.sync.dma_start(
                            out=wq_sb, in_=wq.rearrange("(ec p) f -> p ec f", p=128)
                        )
                        nc.sync.dma_start(
                            out=wk_sb, in_=wk.rearrange("(ec p) f -> p ec f", p=128)
                        )
                    if sc == 2:
                        # prefetch the Q loop's first x chunk during V's tail
                        qk_next = xp.tile(
                            [128, 8, 512], BF16, tag="x", name="qkch0"
                        )
                        nc.sync.dma_start(
                            out=qk_next, in_=qk_xr[0][:, :, 0:512]
                        )

                # Q, K: W stationary, x moving; bias added on eviction
                # (ScalarE).  x chunks are prefetched one ahead (the first
                # was issued during the V loop) so chunk-boundary DMA waits
                # vanish.
                for ci, (ti, wsb, dst) in enumerate(qk_stream):
                    sc = ci % 4
                    ssl = slice(sc * 512, (sc + 1) * 512)
                    xch = qk_next
                    if ci + 1 < len(qk_stream):
                        nti = qk_stream[ci + 1][0]
                        nsc = (ci + 1) % 4
                        qk_next = xp.tile(
                            [128, 8, 512], BF16, tag="x", name=f"qkch{ci+1}"
                        )
                        nc.sync.dma_start(
                            out=qk_next,
                            in_=qk_xr[nti][:, :, nsc * 512 : (nsc + 1) * 512],
                        )
                    if True:
                        for ft in range(4):
                            fsl = slice(ft * 128, (ft + 1) * 128)
                            p = mm1.tile([128, 512], F32, tag="mm")
                            for ec in range(8):
                                nc.tensor.matmul(
                                    p,
                                    wsb[:, ec, fsl],
                                    xch[:, ec, :],
                                    start=(ec == 0),
                                    stop=(ec == 7),
                                )
                            nc.scalar.add(
                                out=dst[:, ft, ssl],
                                in_=p,
                                add=biases[:, ti, ft : ft + 1],
                            )

            # ---------------- P2: attention ----------------
            with tc.tile_pool(name="wp2", bufs=1) as wp2:
              wo_sb = wp2.tile([128, 4, E], BF16, tag="wo")
              nc.sync.dma_start(
                  out=wo_sb, in_=wo.rearrange("(fc p) e -> p fc e", p=128)
              )
              with (
                tc.tile_pool(name="scp", bufs=2, space="PSUM") as scp,
                tc.tile_pool(name="cxp", bufs=2, space="PSUM") as cxp,
              ):
                def scores(pr, qsl, kt):
                    ksl = slice(kt * 128, (kt + 1) * 128)
                    sp = scp.tile([128, 1024], F32, tag="sc", name=f"sp_{kt}")
                    nc.tensor.matmul(
                        sp[:, 0:512],
                        KT[0:64, pr, ksl], QT[0:64, pr, qsl],
                        start=True, stop=True, tile_position=(0, 0),
                    )
                    nc.tensor.matmul(
                        sp[:, 512:1024],
                        KT[64:128, pr, ksl], QT[64:128, pr, qsl],
                        start=True, stop=True, tile_position=(64, 0),
                    )
                    return sp

                def exp_pair(sp, kt, dve_kts):
                    ptt = ptp.tile(
                        [128, 1024], BF16, tag="pt", bufs=16, name=f"pt_{kt}"
                    )
                    if kt in dve_kts:
                        escr = ptp.tile(
                            [128, 1024], F32, tag="escr", bufs=2,
                            name=f"escr_{kt}",
                        )
                        nc.vector._custom_dve(
                            EXPA, out=escr, in0=sp, **EXPA_CONSTS
                        )
                        nc.vector._custom_dve(SQ4, out=ptt, in0=escr)
                    else:
                        nc.scalar.activation(
                            out=ptt, in_=sp, func=AF.Exp, scale=SCALE
                        )
                    return ptt

                norm_pending = []

                def emit_ctx(job):
                    cp, pr, qsl, kt, ptt = job
                    nc.tensor.matmul(
                        cp[:, 0:512], Vn[:, kt, 2 * pr, :], ptt[:, 0:512],
                        start=(kt == 0), stop=(kt == 15),
                    )
                    nc.tensor.matmul(
                        cp[:, 512:1024], Vn[:, kt, 2 * pr + 1, :],
                        ptt[:, 512:1024],
                        start=(kt == 0), stop=(kt == 15),
                    )
                    if kt == 15:
                        # normalize: CX[:, pr, qsl] = ctx / rowsum.
                        # DVE reads PSUM correctly only at partition base 0,
                        # so take the reciprocal of the WHOLE ctx tile (rows
                        # 0-63 are garbage, never read; DVE time is free-dim
                        # bound so this costs the same as a [1,1024] recip) —
                        # row 64 = 1/rowsums.  A tiny sbuf->sbuf DMA (idle
                        # queue) moves it to partition 0 for the broadcast.
                        # This removes the former 1.15us/group ScalarE copy.
                        rc = smp.tile([65, 1024], F32, tag="rc")
                        nc.vector.reciprocal_approx_fast(
                            out=rc, in_=cp[0:65, 0:1024]
                        )
                        inv = smp.tile([1, 1024], F32, tag="inv")
                        nc.sync.dma_start(out=inv, in_=rc[64:65, 0:1024])
                        invB = smp.tile([64, 1024], F32, tag="invB")
                        nc.gpsimd.partition_broadcast(out_ap=invB, in_ap=inv)
                        # the muls wait on the DMA-shift -> broadcast chain
                        # (~3.5us); delay their emission so DVE exp pairs slot
                        # in between instead of queueing behind them
                        norm_pending.append([4, cp, pr, qsl, invB])

                def emit_muls(job):
                    _, cp, pr, qsl, invB = job
                    nc.vector.tensor_mul(
                        CX[0:64, pr, qsl], cp[0:64, 0:512], invB[:, 0:512]
                    )
                    nc.vector.tensor_mul(
                        CX[64:128, pr, qsl], cp[0:64, 512:1024],
                        invB[:, 512:1024]
                    )

                def tick_norms():
                    for job in list(norm_pending):
                        job[0] -= 1
                        if job[0] <= 0:
                            norm_pending.remove(job)
                            emit_muls(job)

                # ctx trails scores/exp by LAG kts: by the time a ctx pair
                # issues, its exp (pt in SBUF, deep pool) finished long ago,
                # so the PE never waits on an in-flight ACT/DVE op.
                LAG = 10
                pending = []
                for qb in range(4):
                    qsl = slice(qb * 512, (qb + 1) * 512)
                    for pr in range(4):
                        g = qb * 4 + pr
                        dve_kts = _DVE_KTS_B if g % 2 == 1 else _DVE_KTS_A
                        cp = cxp.tile([65, 1024], F32, tag="cx")
                        sp_ = scores(pr, qsl, 0)
                        for kt in range(16):
                            ptt = exp_pair(sp_, kt, dve_kts)
                            if kt < 15:
                                sp_ = scores(pr, qsl, kt + 1)
                            pending.append((cp, pr, qsl, kt, ptt))
                            while len(pending) > LAG:
                                emit_ctx(pending.pop(0))
                                tick_norms()
                while pending:
                    emit_ctx(pending.pop(0))
                    tick_norms()
                while norm_pending:
                    emit_muls(norm_pending.pop(0))

              # ---------------- P3: output projection ----------------
              with tc.tile_pool(name="mmo", bufs=4, space="PSUM") as mmo:
                  for qb in range(4):
                      qsl = slice(qb * 512, (qb + 1) * 512)
                      for et in range(8):
                          esl = slice(et * 128, (et + 1) * 128)
                          p = mmo.tile([128, 512], F32, tag="mm")
                          for fc in range(4):
                              nc.tensor.matmul(
                                  p, wo_sb[:, fc, esl], CX[:, fc, qsl],
                                  start=(fc == 0), stop=(fc == 3),
                              )
                          o = ostp.tile([128, 512], BF16, tag="ost")
                          if (qb * 8 + et) % 2 == 0:
                              nc.scalar.copy(out=o, in_=p)
                          else:
                              nc.vector.tensor_copy(out=o, in_=p)
                          nc.sync.dma_start(out=out_d[esl, qsl], in_=o)

    nc.compile()
    _BUILT = nc
    return nc


def _to_bf16(x: np.ndarray):
    import ml_dtypes

    return np.ascontiguousarray(x).astype(ml_dtypes.bfloat16)


def _make_in_maps(inputs):
    query = np.asarray(inputs["query"], dtype=np.float32)
    key_ = np.asarray(inputs["key_"], dtype=np.float32)
    value = np.asarray(inputs["value"], dtype=np.float32)
    Wq = np.asarray(inputs["Wq"], dtype=np.float32)
    bq = np.asarray(inputs["bq"], dtype=np.float32)
    Wk = np.asarray(inputs["Wk"], dtype=np.float32)
    bk = np.asarray(inputs["bk"], dtype=np.float32)
    Wv = np.asarray(inputs["Wv"], dtype=np.float32)
    Wo = np.asarray(inputs["Wo"], dtype=np.float32)

    WqT = _to_bf16(Wq.T)  # [E_in, E_out]
    WkT = _to_bf16(Wk.T)
    WvT = _to_bf16(Wv.T)
    WoT = _to_bf16(Wo.T)  # [F_in, E_out]

    in_maps = []
    for c in range(NCORES):
        b = c // 2
        hh = c % 2
        fsl = slice(hh * F, (hh + 1) * F)
        in_maps.append(
            {
                "xq": _to_bf16(query[b].T),
                "xk": _to_bf16(key_[b].T),
                "xv": _to_bf16(value[b].T),
                "wq": np.ascontiguousarray(WqT[:, fsl]),
                "wk": np.ascontiguousarray(WkT[:, fsl]),
                "wv": np.ascontiguousarray(WvT[:, fsl]),
                "wo": np.ascontiguousarray(WoT[fsl, :]),
                "bq": np.ascontiguousarray(bq[fsl]),
                "bk": np.ascontiguousarray(bk[fsl]),
            }
        )
    return in_maps


def kernel(**inputs) -> np.ndarray:
    from concourse.bass_utils import run_bass_kernel_spmd

    nc = _build_program()
    in_maps = _make_in_maps(inputs)

    bv = np.asarray(inputs["bv"], dtype=np.float32)
    bo = np.asarray(inputs["bo"], dtype=np.float32)
    Wo = np.asarray(inputs["Wo"], dtype=np.float32)
    bo_prime = bo + Wo @ bv  # V-bias folded through softmax + out-proj

    res = run_bass_kernel_spmd(nc, in_maps, core_ids=list(range(NCORES)))

    out = np.empty((B, S, E), dtype=np.float32)
    for b in range(B):
        partial = res.results[2 * b]["out"].astype(np.float32) + res.results[
            2 * b + 1
        ]["out"].astype(np.float32)  # [E, S]
        out[b] = partial.T + bo_prime[None, :]
    return out


# revision 39
# speedup vs baseline: 1.1790x; 1.1790x over previous
"""Multi-head attention (B=4, S=2048, E=1024, H=16) on 8 TRN2 NeuronCores.

Sharding: core c -> (batch b = c//2, head-half hh = c%2  => 8 heads = 512 features).

v4 design (~404us HW, from 485us v2; trace-driven):
 - v2 was exp-bound in P2: 22 exp ACTIVATEs/group @ ~0.72us = 15.8us > PE
   ~14us/group, causing PE stalls, HAM cold-clock oscillation (~127us at
   1.2GHz) and 343us of P2.
 - v3 widens exp to head-PAIRS: both tile-position score halves land in one
   [128,1024] 2-bank PSUM tile; exp is ONE wide op per pair: ACT
   (1024+352)/1.2 = 1.15us (573ns/tile, was 720) for 11-12 pairs/group, DVE
   2-op custom chain over [128,1024] = 2.4us (1.2us/tile, was 1.36) for 4-5
   pairs/group -> ACT ~13.6us, DVE ~14.0us, PE ~13.9us per group: balanced.
 - KEY DECOUPLING: ctx matmuls are emitted LAG=10 kt-slots behind
   scores/exp (pt output buffered in a deep SBUF pool), so by the time a
   ctx pair issues its exp finished long ago -- the PE stopped waiting on
   in-flight ACT/DVE ops, HAM stays warm (cold 127us -> ~27us), and the
   LDWEIGHTS bubbles vanished (P2 median MM cadence 217ns ~= ideal 216).
 - ctx accumulates into one [65,1024] pair tile; softmax denominators: one
   full-tile DVE reciprocal from PSUM base-0 (DVE PSUM reads at partition
   base 64 return garbage -- probed; free-dim-bound so the 65-row recip
   costs the same as [1,1024]; rows 0-63 are unused junk), a tiny
   sbuf->sbuf DMA moves row 64 to partition 0 (idle queue), one wide
   gpsimd partition_broadcast, 2 DVE muls.  This removed the former
   1.15us/group ScalarE rowsum copy that was queue-limiting ACT.
 - A dummy ACTIVATE preloads the exp table set (~2.7us) during P1; first
   input DMAs split so the first V matmul starts ~5us earlier; output
   stored bf16 (halves store DMA; host upcasts and adds the two
   head-half partials).
 - V projection computed TRANSPOSED (x-tile stationary) so V lands directly
   in ctx-stationary layout [keys, head, dk]; V bias folded into host-side
   bo' = bo + Wo @ bv (softmax normalization makes it rowsum-invariant).
"""

import sys

sys.path.insert(0, "/opt/trn_rl_repo")

import numpy as np

B, S, E, H = 4, 2048, 1024, 16
DK = E // H  # 64
NCORES = 8
F = 512  # features per core (head-half)
SCALE = 1.0 / 8.0  # 1/sqrt(DK)

# ---------------------------------------------------------------- helpers

_EXP_OPS = None


def _register_exp_ops():
    """Two custom DVE ops for exp(x/8) on raw scores |x| <= ~28:
    EXPA_ANT: q = (((c3*x + c2)*x + c1)*x + 1)^4  ~= exp(x/128)
    SQ4_ANT:  out = in^16  (4 squarings)  => exp(x/8).
    """
    global _EXP_OPS
    if _EXP_OPS is not None:
        return _EXP_OPS
    import concourse.dve_ops as dve_ops
    from concourse.dve_ops import DveOp, DveOpSpec, get_dve_sub_opcode
    from concourse.dve_spec import Spec, Src0, C0, C1, C2, One, sq, lower

    existing = {op.name: op for op in dve_ops.OPS}
    if "EXPA_ANT" in existing and "SQ4_ANT" in existing:
        _EXP_OPS = (existing["EXPA_ANT"], existing["SQ4_ANT"])
        return _EXP_OPS

    def _ref_a(in0, in1, c0, c1, c2):
        x = in0.astype(np.float32)
        q = ((x * np.float32(c2) + np.float32(c1)) * x + np.float32(c0)) * x + np.float32(1.0)
        q = q * q
        return q * q

    def _ref_sq4(in0, in1, c0, c1, c2):
        x = in0.astype(np.float32)
        for _ in range(4):
            x = x * x
        return x

    opa = DveOp(
        "EXPA_ANT",
        Spec(body=sq(sq(((Src0 * C2 + C1) * Src0 + C0) * Src0 + One)), reference=_ref_a),
        subdim=False,
        uops_sha={},
    )
    opb = DveOp(
        "SQ4_ANT",
        Spec(body=sq(sq(sq(sq(Src0)))), reference=_ref_sq4),
        subdim=False,
        uops_sha={},
    )
    for op in (opa, opb):
        dve_ops.OPS.append(op)
        dve_ops._SUB_OPCODE_FOR_NAME[op.name] = (
            max(dve_ops._SUB_OPCODE_FOR_NAME.values()) + 1
        )
        dve_ops.CUSTOM_DVE_SPECS[op.name] = op.spec
        for ver in ("v3", "v4"):
            try:
                spec_c = DveOpSpec(
                    name=op.name,
                    opcode=get_dve_sub_opcode(op.name),
                    uops=lower(op.spec, ver=ver),
                    rd1_en=False,
                )
                op.uops_sha[ver] = spec_c.sha(ver)
            except Exception:
                pass
    _EXP_OPS = (opa, opb)
    return _EXP_OPS


EXPA_CONSTS = {
    "s0": 1.0 / 512.0,
    "s1": 1.0 / (2.0 * 512.0**2),
    "imm2": 1.0 / (6.0 * 512.0**3),
}

# per-group kts whose exp pair goes to the DVE (2-op wide chain); the rest
# go to ScalarE as one wide ACTIVATE.  Alternating 5/4 DVE pairs balances
# ACT (~1147ns/pair) and DVE (~2384ns/pair + ~2556ns recip+muls) at
# ~13.2us/group each.
_DVE_KTS_A = frozenset({1, 4, 7, 10, 13})
_DVE_KTS_B = frozenset({1, 4, 7, 10})

_BUILT = None  # cached compiled Bass program


def _build_program():
    global _BUILT
    if _BUILT is not None:
        return _BUILT

    import concourse.bass as bass
    import concourse.mybir as mybir
    from concourse import bacc
    from concourse.tile import TileContext

    EXPA, SQ4 = _register_exp_ops()

    F32 = mybir.dt.float32
    BF16 = mybir.dt.bfloat16
    AF = mybir.ActivationFunctionType

    nc = bacc.Bacc("TRN2", target_bir_lowering=False, debug=False, num_devices=NCORES)

    xq = nc.dram_tensor("xq", [E, S], BF16, kind="ExternalInput")
    xk = nc.dram_tensor("xk", [E, S], BF16, kind="ExternalInput")
    xv = nc.dram_tensor("xv", [E, S], BF16, kind="ExternalInput")
    wq = nc.dram_tensor("wq", [E, F], BF16, kind="ExternalInput")
    wk = nc.dram_tensor("wk", [E, F], BF16, kind="ExternalInput")
    wv = nc.dram_tensor("wv", [E, F], BF16, kind="ExternalInput")
    wo = nc.dram_tensor("wo", [F, E], BF16, kind="ExternalInput")
    bq = nc.dram_tensor("bq", [F], F32, kind="ExternalInput")
    bk = nc.dram_tensor("bk", [F], F32, kind="ExternalInput")
    out_d = nc.dram_tensor("out", [E, S], BF16, kind="ExternalOutput")

    with TileContext(nc) as tc:
        with (
            tc.tile_pool(name="persist", bufs=1) as persist,
            tc.tile_pool(name="xp", bufs=2) as xp,
            tc.tile_pool(name="ptp", bufs=4) as ptp,
            tc.tile_pool(name="smp", bufs=2) as smp,
            tc.tile_pool(name="ost", bufs=4) as ostp,
        ):
            QT = persist.tile([128, 4, S], BF16)
            KT = persist.tile([128, 4, S], BF16)
            Vn = persist.tile([128, 16, 8, 65], BF16)
            CX = persist.tile([128, 4, S], BF16)

            # ---------------- P0: ACT exp-table preload ----------
            junk = persist.tile([1, 8], BF16)
            nc.vector.memset(junk, 0.0)
            jexp = persist.tile([1, 8], F32)
            nc.scalar.activation(out=jexp, in_=junk, func=AF.Exp, scale=SCALE)

            # ---------------- P1: projections ----------------
            with (
                tc.tile_pool(name="wp1", bufs=1) as wp1,
                tc.tile_pool(name="mm1", bufs=3, space="PSUM") as mm1,
            ):
                # first DMAs on the queue: what the first matmul needs
                xv_r = xv.rearrange("(ec p) s -> p ec s", p=128)
                xch_next = xp.tile([128, 8, 512], BF16, tag="x", name="xch0")
                wv_sb = wp1.tile([128, 8, F], BF16, tag="wv")
                wv_r = wv.rearrange("(ec p) f -> p ec f", p=128)
                # split the first loads so the first matmul (needs ec=0 only)
                # starts as early as possible
                nc.sync.dma_start(out=xch_next[:, 0:2, :], in_=xv_r[:, 0:2, 0:512])
                nc.sync.dma_start(out=wv_sb[:, 0:2, :], in_=wv_r[:, 0:2, :])
                nc.sync.dma_start(out=xch_next[:, 2:8, :], in_=xv_r[:, 2:8, 0:512])
                nc.sync.dma_start(out=wv_sb[:, 2:8, :], in_=wv_r[:, 2:8, :])
                biases = persist.tile([128, 2, 4], F32)
                for ti, bt in enumerate((bq, bk)):
                    nc.sync.dma_start(
                        out=biases[:, ti, :],
                        in_=bt.rearrange("(ft p) -> p ft", p=128),
                    )
                # ones column for the rowsum trick (V stationary col 64)
                onec = persist.tile([128, 16, 8, 1], F32)
                nc.vector.memset(onec, 1.0)
                nc.vector.tensor_copy(out=Vn[:, :, :, 64:65], in_=onec)

                wq_sb = wp1.tile([128, 8, F], BF16, tag="wq")
                wk_sb = wp1.tile([128, 8, F], BF16, tag="wk")
                qk_xr = (
                    xq.rearrange("(ec p) s -> p ec s", p=128),
                    xk.rearrange("(ec p) s -> p ec s", p=128),
                )
                qk_stream = [(0, wq_sb, QT)] * 4 + [(1, wk_sb, KT)] * 4
                qk_next = None

                # V first: produced transposed ([s, f] = ctx-stationary layout)
                for sc in range(4):
                    xch = xch_next
                    if sc < 3:
                        ssl_n = slice((sc + 1) * 512, (sc + 2) * 512)
                        xch_next = xp.tile(
                            [128, 8, 512], BF16, tag="x", name=f"xch{sc+1}"
                        )
                        nc.sync.dma_start(out=xch_next, in_=xv_r[:, :, ssl_n])
                    for st in range(4):
                        stsl = slice(st * 128, (st + 1) * 128)
                        p = mm1.tile([128, 512], F32, tag="mm")
                        for ec in range(8):
                            nc.tensor.matmul(
                                p,
                                xch[:, ec, stsl],
                                wv_sb[:, ec, :],
                                start=(ec == 0),
                                stop=(ec == 7),
                            )
                        kti = sc * 4 + st
                        nc.vector.tensor_copy(
                            out=Vn[:, kti, :, 0:64],
                            in_=p.rearrange("p (h d) -> p h d", h=8),
                        )
                    if sc == 0:
                        # issue Q/K weight loads while V computes
                        nc.sync.dma_start(
                            out=wq_sb, in_=wq.rearrange("(ec p) f -> p ec f", p=128)
                        )
                        nc.sync.dma_start(
                            out=wk_sb, in_=wk.rearrange("(ec p) f -> p ec f", p=128)
                        )
                    if sc == 2:
                        # prefetch the Q loop's first x chunk during V's tail
                        qk_next = xp.tile(
                            [128, 8, 512], BF16, tag="x", name="qkch0"
                        )
                        nc.sync.dma_start(
                            out=qk_next, in_=qk_xr[0][:, :, 0:512]
                        )

                # Q, K: W stationary, x moving; bias added on eviction
                # (ScalarE).  x chunks are prefetched one ahead (the first
                # was issued during the V loop) so chunk-boundary DMA waits
                # vanish.
                for ci, (ti, wsb, dst) in enumerate(qk_stream):
                    sc = ci % 4
                    ssl = slice(sc * 512, (sc + 1) * 512)
                    xch = qk_next
                    if ci + 1 < len(qk_stream):
                        nti = qk_stream[ci + 1][0]
                        nsc = (ci + 1) % 4
                        qk_next = xp.tile(
                            [128, 8, 512], BF16, tag="x", name=f"qkch{ci+1}"
                        )
                        nc.sync.dma_start(
                            out=qk_next,
                            in_=qk_xr[nti][:, :, nsc * 512 : (nsc + 1) * 512],
                        )
                    if True:
                        for ft in range(4):
                            fsl = slice(ft * 128, (ft + 1) * 128)
                            p = mm1.tile([128, 512], F32, tag="mm")
                            for ec in range(8):
                                nc.tensor.matmul(
                                    p,
                                    wsb[:, ec, fsl],
                                    xch[:, ec, :],
                                    start=(ec == 0),
                                    stop=(ec == 7),
                                )
                            nc.scalar.add(
                                out=dst[:, ft, ssl],
                                in_=p,
                                add=biases[:, ti, ft : ft + 1],
                            )

            # ---------------- P2: attention ----------------
            with tc.tile_pool(name="wp2", bufs=1) as wp2:
              wo_sb = wp2.tile([128, 4, E], BF16, tag="wo")
              nc.sync.dma_start(
                  out=wo_sb, in_=wo.rearrange("(fc p) e -> p fc e", p=128)
              )
              with (
                tc.tile_pool(name="scp", bufs=2, space="PSUM") as scp,
                tc.tile_pool(name="cxp", bufs=2, space="PSUM") as cxp,
              ):
                def scores(pr, qsl, kt):
                    ksl = slice(kt * 128, (kt + 1) * 128)
                    sp = scp.tile([128, 1024], F32, tag="sc", name=f"sp_{kt}")
                    nc.tensor.matmul(
                        sp[:, 0:512],
                        KT[0:64, pr, ksl], QT[0:64, pr, qsl],
                        start=True, stop=True, tile_position=(0, 0),
                    )
                    nc.tensor.matmul(
                        sp[:, 512:1024],
                        KT[64:128, pr, ksl], QT[64:128, pr, qsl],
                        start=True, stop=True, tile_position=(64, 0),
                    )
                    return sp

                def exp_pair(sp, kt, dve_kts):
                    ptt = ptp.tile(
                        [128, 1024], BF16, tag="pt", bufs=16, name=f"pt_{kt}"
                    )
                    if kt in dve_kts:
                        escr = ptp.tile(
                            [128, 1024], F32, tag="escr", bufs=2,
                            name=f"escr_{kt}",
                        )
                        nc.vector._custom_dve(
                            EXPA, out=escr, in0=sp, **EXPA_CONSTS
                        )
                        nc.vector._custom_dve(SQ4, out=ptt, in0=escr)
                    else:
                        nc.scalar.activation(
                            out=ptt, in_=sp, func=AF.Exp, scale=SCALE
                        )
                    return ptt

                norm_pending = []

                def emit_ctx(job):
                    cp, pr, qsl, kt, ptt = job
                    nc.tensor.matmul(
                        cp[:, 0:512], Vn[:, kt, 2 * pr, :], ptt[:, 0:512],
                        start=(kt == 0), stop=(kt == 15),
                    )
                    nc.tensor.matmul(
                        cp[:, 512:1024], Vn[:, kt, 2 * pr + 1, :],
                        ptt[:, 512:1024],
                        start=(kt == 0), stop=(kt == 15),
                    )
                    if kt == 15:
                        # normalize: CX[:, pr, qsl] = ctx / rowsum.
                        # DVE reads PSUM correctly only at partition base 0,
                        # so take the reciprocal of the WHOLE ctx tile (rows
                        # 0-63 are garbage, never read; DVE time is free-dim
                        # bound so this costs the same as a [1,1024] recip) —
                        # row 64 = 1/rowsums.  A tiny sbuf->sbuf DMA (idle
                        # queue) moves it to partition 0 for the broadcast.
                        # This removes the former 1.15us/group ScalarE copy.
                        rc = smp.tile([65, 1024], F32, tag="rc")
                        nc.vector.reciprocal_approx_fast(
                            out=rc, in_=cp[0:65, 0:1024]
                        )
                        inv = smp.tile([1, 1024], F32, tag="inv")
                        nc.sync.dma_start(out=inv, in_=rc[64:65, 0:1024])
                        invB = smp.tile([64, 1024], F32, tag="invB")
                        nc.gpsimd.partition_broadcast(out_ap=invB, in_ap=inv)
                        # the muls wait on the DMA-shift -> broadcast chain
                        # (~3.5us); delay their emission so DVE exp pairs slot
                        # in between instead of queueing behind them
                        norm_pending.append([4, cp, pr, qsl, invB])

                def emit_muls(job):
                    _, cp, pr, qsl, invB = job
                    nc.vector.tensor_mul(
                        CX[0:64, pr, qsl], cp[0:64, 0:512], invB[:, 0:512]
                    )
                    nc.vector.tensor_mul(
                        CX[64:128, pr, qsl], cp[0:64, 512:1024],
                        invB[:, 512:1024]
                    )

                def tick_norms():
                    for job in list(norm_pending):
                        job[0] -= 1
                        if job[0] <= 0:
                            norm_pending.remove(job)
                            emit_muls(job)

                # ctx trails scores/exp by LAG kts: by the time a ctx pair
                # issues, its exp (pt in SBUF, deep pool) finished long ago,
                # so the PE never waits on an in-flight ACT/DVE op.
                LAG = 10
                pending = []
                for qb in range(4):
                    qsl = slice(qb * 512, (qb + 1) * 512)
                    for pr in range(4):
                        g = qb * 4 + pr
                        dve_kts = _DVE_KTS_B if g % 2 == 1 else _DVE_KTS_A
                        cp = cxp.tile([65, 1024], F32, tag="cx")
                        sp_ = scores(pr, qsl, 0)
                        for kt in range(16):
                            ptt = exp_pair(sp_, kt, dve_kts)
                            if kt < 15:
                                sp_ = scores(pr, qsl, kt + 1)
                            pending.append((cp, pr, qsl, kt, ptt))
                            while len(pending) > LAG:
                                emit_ctx(pending.pop(0))
                                tick_norms()
                while pending:
                    emit_ctx(pending.pop(0))
                    tick_norms()
                while norm_pending:
                    emit_muls(norm_pending.pop(0))

              # ---------------- P3: output projection ----------------
              with tc.tile_pool(name="mmo", bufs=4, space="PSUM") as mmo:
                  for qb in range(4):
                      qsl = slice(qb * 512, (qb + 1) * 512)
                      for et in range(8):
                          esl = slice(et * 128, (et + 1) * 128)
                          p = mmo.tile([128, 512], F32, tag="mm")
                          for fc in range(4):
                              nc.tensor.matmul(
                                  p, wo_sb[:, fc, esl], CX[:, fc, qsl],
                                  start=(fc == 0), stop=(fc == 3),
                              )
                          o = ostp.tile([128, 512], BF16, tag="ost")
                          if (qb * 8 + et) % 2 == 0:
                              nc.scalar.copy(out=o, in_=p)
                          else:
                              nc.vector.tensor_copy(out=o, in_=p)
                          nc.sync.dma_start(out=out_d[esl, qsl], in_=o)

    nc.compile()
    _BUILT = nc
    return nc


def _to_bf16(x: np.ndarray):
    import ml_dtypes

    return np.ascontiguousarray(x).astype(ml_dtypes.bfloat16)


def _make_in_maps(inputs):
    query = np.asarray(inputs["query"], dtype=np.float32)
    key_ = np.asarray(inputs["key_"], dtype=np.float32)
    value = np.asarray(inputs["value"], dtype=np.float32)
    Wq = np.asarray(inputs["Wq"], dtype=np.float32)
    bq = np.asarray(inputs["bq"], dtype=np.float32)
    Wk = np.asarray(inputs["Wk"], dtype=np.float32)
    bk = np.asarray(inputs["bk"], dtype=np.float32)
    Wv = np.asarray(inputs["Wv"], dtype=np.float32)
    Wo = np.asarray(inputs["Wo"], dtype=np.float32)

    WqT = _to_bf16(Wq.T)  # [E_in, E_out]
    WkT = _to_bf16(Wk.T)
    WvT = _to_bf16(Wv.T)
    WoT = _to_bf16(Wo.T)  # [F_in, E_out]

    in_maps = []
    for c in range(NCORES):
        b = c // 2
        hh = c % 2
        fsl = slice(hh * F, (hh + 1) * F)
        in_maps.append(
            {
                "xq": _to_bf16(query[b].T),
                "xk": _to_bf16(key_[b].T),
                "xv": _to_bf16(value[b].T),
                "wq": np.ascontiguousarray(WqT[:, fsl]),
                "wk": np.ascontiguousarray(WkT[:, fsl]),
                "wv": np.ascontiguousarray(WvT[:, fsl]),
                "wo": np.ascontiguousarray(WoT[fsl, :]),
                "bq": np.ascontiguousarray(bq[fsl]),
                "bk": np.ascontiguousarray(bk[fsl]),
            }
        )
    return in_maps


def kernel(**inputs) -> np.ndarray:
    from concourse.bass_utils import run_bass_kernel_spmd

    nc = _build_program()
    in_maps = _make_in_maps(inputs)

    bv = np.asarray(inputs["bv"], dtype=np.float32)
    bo = np.asarray(inputs["bo"], dtype=np.float32)
    Wo = np.asarray(inputs["Wo"], dtype=np.float32)
    bo_prime = bo + Wo @ bv  # V-bias folded through softmax + out-proj

    res = run_bass_kernel_spmd(nc, in_maps, core_ids=list(range(NCORES)))

    out = np.empty((B, S, E), dtype=np.float32)
    for b in range(B):
        partial = res.results[2 * b]["out"].astype(np.float32) + res.results[
            2 * b + 1
        ]["out"].astype(np.float32)  # [E, S]
        out[b] = partial.T + bo_prime[None, :]
    return out


# revision 40
# speedup vs baseline: 1.1860x; 1.0059x over previous
"""Multi-head attention (B=4, S=2048, E=1024, H=16) on 8 TRN2 NeuronCores.

Sharding: core c -> (batch b = c//2, head-half hh = c%2  => 8 heads = 512 features).

v4 design (~404us HW, from 485us v2; trace-driven):
 - v2 was exp-bound in P2: 22 exp ACTIVATEs/group @ ~0.72us = 15.8us > PE
   ~14us/group, causing PE stalls, HAM cold-clock oscillation (~127us at
   1.2GHz) and 343us of P2.
 - v3 widens exp to head-PAIRS: both tile-position score halves land in one
   [128,1024] 2-bank PSUM tile; exp is ONE wide op per pair: ACT
   (1024+352)/1.2 = 1.15us (573ns/tile, was 720) for 11-12 pairs/group, DVE
   2-op custom chain over [128,1024] = 2.4us (1.2us/tile, was 1.36) for 4-5
   pairs/group -> ACT ~13.6us, DVE ~14.0us, PE ~13.9us per group: balanced.
 - KEY DECOUPLING: ctx matmuls are emitted LAG=10 kt-slots behind
   scores/exp (pt output buffered in a deep SBUF pool), so by the time a
   ctx pair issues its exp finished long ago -- the PE stopped waiting on
   in-flight ACT/DVE ops, HAM stays warm (cold 127us -> ~27us), and the
   LDWEIGHTS bubbles vanished (P2 median MM cadence 217ns ~= ideal 216).
 - ctx accumulates into one [65,1024] pair tile; softmax denominators: one
   full-tile DVE reciprocal from PSUM base-0 (DVE PSUM reads at partition
   base 64 return garbage -- probed; free-dim-bound so the 65-row recip
   costs the same as [1,1024]; rows 0-63 are unused junk), a tiny
   sbuf->sbuf DMA moves row 64 to partition 0 (idle queue), one wide
   gpsimd partition_broadcast, 2 DVE muls.  This removed the former
   1.15us/group ScalarE rowsum copy that was queue-limiting ACT.
 - A dummy ACTIVATE preloads the exp table set (~2.7us) during P1; first
   input DMAs split so the first V matmul starts ~5us earlier; output
   stored bf16 (halves store DMA; host upcasts and adds the two
   head-half partials).
 - V projection computed TRANSPOSED (x-tile stationary) so V lands directly
   in ctx-stationary layout [keys, head, dk]; V bias folded into host-side
   bo' = bo + Wo @ bv (softmax normalization makes it rowsum-invariant).
"""

import sys

sys.path.insert(0, "/opt/trn_rl_repo")

import numpy as np

B, S, E, H = 4, 2048, 1024, 16
DK = E // H  # 64
NCORES = 8
F = 512  # features per core (head-half)
SCALE = 1.0 / 8.0  # 1/sqrt(DK)

# ---------------------------------------------------------------- helpers

_EXP_OPS = None


def _register_exp_ops():
    """Two custom DVE ops for exp(x/8) on raw scores |x| <= ~28:
    EXPA_ANT: q = (((c3*x + c2)*x + c1)*x + 1)^4  ~= exp(x/128)
    SQ4_ANT:  out = in^16  (4 squarings)  => exp(x/8).
    """
    global _EXP_OPS
    if _EXP_OPS is not None:
        return _EXP_OPS
    import concourse.dve_ops as dve_ops
    from concourse.dve_ops import DveOp, DveOpSpec, get_dve_sub_opcode
    from concourse.dve_spec import Spec, Src0, C0, C1, C2, One, sq, lower

    existing = {op.name: op for op in dve_ops.OPS}
    if "EXPA_ANT" in existing and "SQ4_ANT" in existing:
        _EXP_OPS = (existing["EXPA_ANT"], existing["SQ4_ANT"])
        return _EXP_OPS

    def _ref_a(in0, in1, c0, c1, c2):
        x = in0.astype(np.float32)
        q = ((x * np.float32(c2) + np.float32(c1)) * x + np.float32(c0)) * x + np.float32(1.0)
        q = q * q
        return q * q

    def _ref_sq4(in0, in1, c0, c1, c2):
        x = in0.astype(np.float32)
        for _ in range(4):
            x = x * x
        return x

    opa = DveOp(
        "EXPA_ANT",
        Spec(body=sq(sq(((Src0 * C2 + C1) * Src0 + C0) * Src0 + One)), reference=_ref_a),
        subdim=False,
        uops_sha={},
    )
    opb = DveOp(
        "SQ4_ANT",
        Spec(body=sq(sq(sq(sq(Src0)))), reference=_ref_sq4),
        subdim=False,
        uops_sha={},
    )
    for op in (opa, opb):
        dve_ops.OPS.append(op)
        dve_ops._SUB_OPCODE_FOR_NAME[op.name] = (
            max(dve_ops._SUB_OPCODE_FOR_NAME.values()) + 1
        )
        dve_ops.CUSTOM_DVE_SPECS[op.name] = op.spec
        for ver in ("v3", "v4"):
            try:
                spec_c = DveOpSpec(
                    name=op.name,
                    opcode=get_dve_sub_opcode(op.name),
                    uops=lower(op.spec, ver=ver),
                    rd1_en=False,
                )
                op.uops_sha[ver] = spec_c.sha(ver)
            except Exception:
                pass
    _EXP_OPS = (opa, opb)
    return _EXP_OPS


EXPA_CONSTS = {
    "s0": 1.0 / 512.0,
    "s1": 1.0 / (2.0 * 512.0**2),
    "imm2": 1.0 / (6.0 * 512.0**3),
}

# per-group kts whose exp pair goes to the DVE (2-op wide chain); the rest
# go to ScalarE as one wide ACTIVATE.  Alternating 5/4 DVE pairs balances
# ACT (~1147ns/pair) and DVE (~2384ns/pair + ~2556ns recip+muls) at
# ~13.2us/group each.
_DVE_KTS_A = frozenset({1, 4, 7, 10, 13})
_DVE_KTS_B = frozenset({1, 4, 7, 10})

_BUILT = None  # cached compiled Bass program


def _build_program():
    global _BUILT
    if _BUILT is not None:
        return _BUILT

    import concourse.bass as bass
    import concourse.mybir as mybir
    from concourse import bacc
    from concourse.tile import TileContext

    EXPA, SQ4 = _register_exp_ops()

    F32 = mybir.dt.float32
    BF16 = mybir.dt.bfloat16
    AF = mybir.ActivationFunctionType

    nc = bacc.Bacc("TRN2", target_bir_lowering=False, debug=False, num_devices=NCORES)

    xq = nc.dram_tensor("xq", [E, S], BF16, kind="ExternalInput")
    xk = nc.dram_tensor("xk", [E, S], BF16, kind="ExternalInput")
    xv = nc.dram_tensor("xv", [E, S], BF16, kind="ExternalInput")
    wq = nc.dram_tensor("wq", [E, F], BF16, kind="ExternalInput")
    wk = nc.dram_tensor("wk", [E, F], BF16, kind="ExternalInput")
    wv = nc.dram_tensor("wv", [E, F], BF16, kind="ExternalInput")
    wo = nc.dram_tensor("wo", [F, E], BF16, kind="ExternalInput")
    bq = nc.dram_tensor("bq", [F], F32, kind="ExternalInput")
    bk = nc.dram_tensor("bk", [F], F32, kind="ExternalInput")
    out_d = nc.dram_tensor("out", [E, S], BF16, kind="ExternalOutput")

    with TileContext(nc) as tc:
        with (
            tc.tile_pool(name="persist", bufs=1) as persist,
            tc.tile_pool(name="xp", bufs=2) as xp,
            tc.tile_pool(name="ptp", bufs=4) as ptp,
            tc.tile_pool(name="smp", bufs=2) as smp,
            tc.tile_pool(name="ost", bufs=4) as ostp,
        ):
            QT = persist.tile([128, 4, S], BF16)
            KT = persist.tile([128, 4, S], BF16)
            Vn = persist.tile([128, 16, 8, 65], BF16)
            CX = persist.tile([128, 4, S], BF16)

            # ---------------- P0: ACT exp-table preload ----------
            junk = persist.tile([1, 8], BF16)
            nc.vector.memset(junk, 0.0)
            jexp = persist.tile([1, 8], F32)
            nc.scalar.activation(out=jexp, in_=junk, func=AF.Exp, scale=SCALE)

            # ---------------- P1: projections ----------------
            with (
                tc.tile_pool(name="wp1", bufs=1) as wp1,
                tc.tile_pool(name="mm1", bufs=3, space="PSUM") as mm1,
            ):
                # first DMAs on the queue: what the first matmul needs
                xv_r = xv.rearrange("(ec p) s -> p ec s", p=128)
                xch_next = xp.tile([128, 8, 512], BF16, tag="x", name="xch0")
                wv_sb = wp1.tile([128, 8, F], BF16, tag="wv")
                wv_r = wv.rearrange("(ec p) f -> p ec f", p=128)
                # split the first loads so the first matmul (needs ec=0 only)
                # starts as early as possible
                nc.sync.dma_start(out=xch_next[:, 0:2, :], in_=xv_r[:, 0:2, 0:512])
                nc.sync.dma_start(out=wv_sb[:, 0:2, :], in_=wv_r[:, 0:2, :])
                nc.sync.dma_start(out=xch_next[:, 2:8, :], in_=xv_r[:, 2:8, 0:512])
                nc.sync.dma_start(out=wv_sb[:, 2:8, :], in_=wv_r[:, 2:8, :])
                biases = persist.tile([128, 2, 4], F32)
                for ti, bt in enumerate((bq, bk)):
                    nc.sync.dma_start(
                        out=biases[:, ti, :],
                        in_=bt.rearrange("(ft p) -> p ft", p=128),
                    )
                # ones column for the rowsum trick (V stationary col 64)
                onec = persist.tile([128, 16, 8, 1], F32)
                nc.vector.memset(onec, 1.0)
                nc.vector.tensor_copy(out=Vn[:, :, :, 64:65], in_=onec)

                wq_sb = wp1.tile([128, 8, F], BF16, tag="wq")
                wk_sb = wp1.tile([128, 8, F], BF16, tag="wk")
                qk_xr = (
                    xq.rearrange("(ec p) s -> p ec s", p=128),
                    xk.rearrange("(ec p) s -> p ec s", p=128),
                )
                qk_stream = [(0, wq_sb, QT)] * 4 + [(1, wk_sb, KT)] * 4
                qk_next = None

                # V first: produced transposed ([s, f] = ctx-stationary layout)
                for sc in range(4):
                    xch = xch_next
                    if sc < 3:
                        ssl_n = slice((sc + 1) * 512, (sc + 2) * 512)
                        xch_next = xp.tile(
                            [128, 8, 512], BF16, tag="x", name=f"xch{sc+1}"
                        )
                        nc.sync.dma_start(out=xch_next, in_=xv_r[:, :, ssl_n])
                    for st in range(4):
                        stsl = slice(st * 128, (st + 1) * 128)
                        p = mm1.tile([128, 512], F32, tag="mm")
                        for ec in range(8):
                            nc.tensor.matmul(
                                p,
                                xch[:, ec, stsl],
                                wv_sb[:, ec, :],
                                start=(ec == 0),
                                stop=(ec == 7),
                            )
                        kti = sc * 4 + st
                        nc.vector.tensor_copy(
                            out=Vn[:, kti, :, 0:64],
                            in_=p.rearrange("p (h d) -> p h d", h=8),
                        )
                    if sc == 0:
                        # issue Q/K weight loads while V computes
                        nc.sync.dma_start(
                            out=wq_sb, in_=wq.rearrange("(ec p) f -> p ec f", p=128)
                        )
                        nc.sync.dma_start(
                            out=wk_sb, in_=wk.rearrange("(ec p) f -> p ec f", p=128)
                        )
                    if sc == 2:
                        # prefetch the Q loop's first x chunk during V's tail
                        qk_next = xp.tile(
                            [128, 8, 512], BF16, tag="x", name="qkch0"
                        )
                        nc.sync.dma_start(
                            out=qk_next, in_=qk_xr[0][:, :, 0:512]
                        )

                # Q, K: W stationary, x moving; bias added on eviction
                # (ScalarE).  x chunks are prefetched one ahead (the first
                # was issued during the V loop) so chunk-boundary DMA waits
                # vanish.
                for ci, (ti, wsb, dst) in enumerate(qk_stream):
                    sc = ci % 4
                    ssl = slice(sc * 512, (sc + 1) * 512)
                    xch = qk_next
                    if ci + 1 < len(qk_stream):
                        nti = qk_stream[ci + 1][0]
                        nsc = (ci + 1) % 4
                        qk_next = xp.tile(
                            [128, 8, 512], BF16, tag="x", name=f"qkch{ci+1}"
                        )
                        nc.sync.dma_start(
                            out=qk_next,
                            in_=qk_xr[nti][:, :, nsc * 512 : (nsc + 1) * 512],
                        )
                    if True:
                        for ft in range(4):
                            fsl = slice(ft * 128, (ft + 1) * 128)
                            p = mm1.tile([128, 512], F32, tag="mm")
                            for ec in range(8):
                                nc.tensor.matmul(
                                    p,
                                    wsb[:, ec, fsl],
                                    xch[:, ec, :],
                                    start=(ec == 0),
                                    stop=(ec == 7),
                                )
                            nc.scalar.add(
                                out=dst[:, ft, ssl],
                                in_=p,
                                add=biases[:, ti, ft : ft + 1],
                            )

            # ---------------- P2: attention ----------------
            with tc.tile_pool(name="wp2", bufs=1) as wp2:
              wo_sb = wp2.tile([128, 4, E], BF16, tag="wo")
              nc.sync.dma_start(
                  out=wo_sb, in_=wo.rearrange("(fc p) e -> p fc e", p=128)
              )
              with (
                tc.tile_pool(name="scp", bufs=2, space="PSUM") as scp,
                tc.tile_pool(name="cxp", bufs=2, space="PSUM") as cxp,
              ):
                def scores(pr, qsl, kt):
                    ksl = slice(kt * 128, (kt + 1) * 128)
                    sp = scp.tile([128, 1024], F32, tag="sc", name=f"sp_{kt}")
                    nc.tensor.matmul(
                        sp[:, 0:512],
                        KT[0:64, pr, ksl], QT[0:64, pr, qsl],
                        start=True, stop=True, tile_position=(0, 0),
                    )
                    nc.tensor.matmul(
                        sp[:, 512:1024],
                        KT[64:128, pr, ksl], QT[64:128, pr, qsl],
                        start=True, stop=True, tile_position=(64, 0),
                    )
                    return sp

                def exp_pair(sp, kt, dve_kts):
                    ptt = ptp.tile(
                        [128, 1024], BF16, tag="pt", bufs=13, name=f"pt_{kt}"
                    )
                    if kt in dve_kts:
                        escr = ptp.tile(
                            [128, 1024], F32, tag="escr", bufs=2,
                            name=f"escr_{kt}",
                        )
                        nc.vector._custom_dve(
                            EXPA, out=escr, in0=sp, **EXPA_CONSTS
                        )
                        nc.vector._custom_dve(SQ4, out=ptt, in0=escr)
                    else:
                        nc.scalar.activation(
                            out=ptt, in_=sp, func=AF.Exp, scale=SCALE
                        )
                    return ptt

                norm_pending = []

                def emit_ctx(job):
                    cp, pr, qsl, kt, ptt = job
                    nc.tensor.matmul(
                        cp[:, 0:512], Vn[:, kt, 2 * pr, :], ptt[:, 0:512],
                        start=(kt == 0), stop=(kt == 15),
                    )
                    nc.tensor.matmul(
                        cp[:, 512:1024], Vn[:, kt, 2 * pr + 1, :],
                        ptt[:, 512:1024],
                        start=(kt == 0), stop=(kt == 15),
                    )
                    if kt == 15:
                        # normalize: CX[:, pr, qsl] = ctx / rowsum.
                        # DVE reads PSUM correctly only at partition base 0,
                        # so take the reciprocal of the WHOLE ctx tile (rows
                        # 0-63 are garbage, never read; DVE time is free-dim
                        # bound so this costs the same as a [1,1024] recip) —
                        # row 64 = 1/rowsums.  A tiny sbuf->sbuf DMA (idle
                        # queue) moves it to partition 0 for the broadcast.
                        # This removes the former 1.15us/group ScalarE copy.
                        rc = smp.tile([65, 1024], F32, tag="rc")
                        nc.vector.reciprocal_approx_fast(
                            out=rc, in_=cp[0:65, 0:1024]
                        )
                        inv = smp.tile([1, 1024], F32, tag="inv")
                        nc.sync.dma_start(out=inv, in_=rc[64:65, 0:1024])
                        invB = smp.tile([64, 1024], F32, tag="invB")
                        nc.gpsimd.partition_broadcast(out_ap=invB, in_ap=inv)
                        # the muls wait on the DMA-shift -> broadcast chain
                        # (~3.5us); delay their emission so DVE exp pairs slot
                        # in between instead of queueing behind them
                        norm_pending.append([4, cp, pr, qsl, invB])

                def emit_muls(job):
                    _, cp, pr, qsl, invB = job
                    nc.vector.tensor_mul(
                        CX[0:64, pr, qsl], cp[0:64, 0:512], invB[:, 0:512]
                    )
                    nc.vector.tensor_mul(
                        CX[64:128, pr, qsl], cp[0:64, 512:1024],
                        invB[:, 512:1024]
                    )

                def tick_norms():
                    for job in list(norm_pending):
                        job[0] -= 1
                        if job[0] <= 0:
                            norm_pending.remove(job)
                            emit_muls(job)

                # ctx trails scores/exp by LAG kts: by the time a ctx pair
                # issues, its exp (pt in SBUF, deep pool) finished long ago,
                # so the PE never waits on an in-flight ACT/DVE op.
                LAG = 7
                pending = []
                for qb in range(4):
                    qsl = slice(qb * 512, (qb + 1) * 512)
                    for pr in range(4):
                        g = qb * 4 + pr
                        dve_kts = _DVE_KTS_B if g % 2 == 1 else _DVE_KTS_A
                        cp = cxp.tile([65, 1024], F32, tag="cx")
                        sp_ = scores(pr, qsl, 0)
                        for kt in range(16):
                            ptt = exp_pair(sp_, kt, dve_kts)
                            if kt < 15:
                                sp_ = scores(pr, qsl, kt + 1)
                            pending.append((cp, pr, qsl, kt, ptt))
                            while len(pending) > LAG:
                                emit_ctx(pending.pop(0))
                                tick_norms()
                while pending:
                    emit_ctx(pending.pop(0))
                    tick_norms()
                while norm_pending:
                    emit_muls(norm_pending.pop(0))

              # ---------------- P3: output projection ----------------
              with tc.tile_pool(name="mmo", bufs=4, space="PSUM") as mmo:
                  for qb in range(4):
                      qsl = slice(qb * 512, (qb + 1) * 512)
                      for et in range(8):
                          esl = slice(et * 128, (et + 1) * 128)
                          p = mmo.tile([128, 512], F32, tag="mm")
                          for fc in range(4):
                              nc.tensor.matmul(
                                  p, wo_sb[:, fc, esl], CX[:, fc, qsl],
                                  start=(fc == 0), stop=(fc == 3),
                              )
                          o = ostp.tile([128, 512], BF16, tag="ost")
                          if (qb * 8 + et) % 2 == 0:
                              nc.scalar.copy(out=o, in_=p)
                          else:
                              nc.vector.tensor_copy(out=o, in_=p)
                          nc.sync.dma_start(out=out_d[esl, qsl], in_=o)

    nc.compile()
    _BUILT = nc
    return nc


def _to_bf16(x: np.ndarray):
    import ml_dtypes

    return np.ascontiguousarray(x).astype(ml_dtypes.bfloat16)


def _make_in_maps(inputs):
    query = np.asarray(inputs["query"], dtype=np.float32)
    key_ = np.asarray(inputs["key_"], dtype=np.float32)
    value = np.asarray(inputs["value"], dtype=np.float32)
    Wq = np.asarray(inputs["Wq"], dtype=np.float32)
    bq = np.asarray(inputs["bq"], dtype=np.float32)
    Wk = np.asarray(inputs["Wk"], dtype=np.float32)
    bk = np.asarray(inputs["bk"], dtype=np.float32)
    Wv = np.asarray(inputs["Wv"], dtype=np.float32)
    Wo = np.asarray(inputs["Wo"], dtype=np.float32)

    WqT = _to_bf16(Wq.T)  # [E_in, E_out]
    WkT = _to_bf16(Wk.T)
    WvT = _to_bf16(Wv.T)
    WoT = _to_bf16(Wo.T)  # [F_in, E_out]

    in_maps = []
    for c in range(NCORES):
        b = c // 2
        hh = c % 2
        fsl = slice(hh * F, (hh + 1) * F)
        in_maps.append(
            {
                "xq": _to_bf16(query[b].T),
                "xk": _to_bf16(key_[b].T),
                "xv": _to_bf16(value[b].T),
                "wq": np.ascontiguousarray(WqT[:, fsl]),
                "wk": np.ascontiguousarray(WkT[:, fsl]),
                "wv": np.ascontiguousarray(WvT[:, fsl]),
                "wo": np.ascontiguousarray(WoT[fsl, :]),
                "bq": np.ascontiguousarray(bq[fsl]),
                "bk": np.ascontiguousarray(bk[fsl]),
            }
        )
    return in_maps


def kernel(**inputs) -> np.ndarray:
    from concourse.bass_utils import run_bass_kernel_spmd

    nc = _build_program()
    in_maps = _make_in_maps(inputs)

    bv = np.asarray(inputs["bv"], dtype=np.float32)
    bo = np.asarray(inputs["bo"], dtype=np.float32)
    Wo = np.asarray(inputs["Wo"], dtype=np.float32)
    bo_prime = bo + Wo @ bv  # V-bias folded through softmax + out-proj

    res = run_bass_kernel_spmd(nc, in_maps, core_ids=list(range(NCORES)))

    out = np.empty((B, S, E), dtype=np.float32)
    for b in range(B):
        partial = res.results[2 * b]["out"].astype(np.float32) + res.results[
            2 * b + 1
        ]["out"].astype(np.float32)  # [E, S]
        out[b] = partial.T + bo_prime[None, :]
    return out


# revision 42
# speedup vs baseline: 1.2009x; 1.0126x over previous
"""Multi-head attention (B=4, S=2048, E=1024, H=16) on 8 TRN2 NeuronCores.

Sharding: core c -> (batch b = c//2, head-half hh = c%2  => 8 heads = 512 features).

v4 design (~404us HW, from 485us v2; trace-driven):
 - v2 was exp-bound in P2: 22 exp ACTIVATEs/group @ ~0.72us = 15.8us > PE
   ~14us/group, causing PE stalls, HAM cold-clock oscillation (~127us at
   1.2GHz) and 343us of P2.
 - v3 widens exp to head-PAIRS: both tile-position score halves land in one
   [128,1024] 2-bank PSUM tile; exp is ONE wide op per pair: ACT
   (1024+352)/1.2 = 1.15us (573ns/tile, was 720) for 11-12 pairs/group, DVE
   2-op custom chain over [128,1024] = 2.4us (1.2us/tile, was 1.36) for 4-5
   pairs/group -> ACT ~13.6us, DVE ~14.0us, PE ~13.9us per group: balanced.
 - KEY DECOUPLING: ctx matmuls are emitted LAG=10 kt-slots behind
   scores/exp (pt output buffered in a deep SBUF pool), so by the time a
   ctx pair issues its exp finished long ago -- the PE stopped waiting on
   in-flight ACT/DVE ops, HAM stays warm (cold 127us -> ~27us), and the
   LDWEIGHTS bubbles vanished (P2 median MM cadence 217ns ~= ideal 216).
 - ctx accumulates into one [65,1024] pair tile; softmax denominators: one
   full-tile DVE reciprocal from PSUM base-0 (DVE PSUM reads at partition
   base 64 return garbage -- probed; free-dim-bound so the 65-row recip
   costs the same as [1,1024]; rows 0-63 are unused junk), a tiny
   sbuf->sbuf DMA moves row 64 to partition 0 (idle queue), one wide
   gpsimd partition_broadcast, 2 DVE muls.  This removed the former
   1.15us/group ScalarE rowsum copy that was queue-limiting ACT.
 - A dummy ACTIVATE preloads the exp table set (~2.7us) during P1; first
   input DMAs split so the first V matmul starts ~5us earlier; output
   stored bf16 (halves store DMA; host upcasts and adds the two
   head-half partials).
 - V projection computed TRANSPOSED (x-tile stationary) so V lands directly
   in ctx-stationary layout [keys, head, dk]; V bias folded into host-side
   bo' = bo + Wo @ bv (softmax normalization makes it rowsum-invariant).
"""

import sys

sys.path.insert(0, "/opt/trn_rl_repo")

import numpy as np

B, S, E, H = 4, 2048, 1024, 16
DK = E // H  # 64
NCORES = 8
F = 512  # features per core (head-half)
SCALE = 1.0 / 8.0  # 1/sqrt(DK)

# ---------------------------------------------------------------- helpers

_EXP_OPS = None


def _register_exp_ops():
    """Two custom DVE ops for exp(x/8) on raw scores |x| <= ~28:
    EXPA_ANT: q = (((c3*x + c2)*x + c1)*x + 1)^4  ~= exp(x/128)
    SQ4_ANT:  out = in^16  (4 squarings)  => exp(x/8).
    """
    global _EXP_OPS
    if _EXP_OPS is not None:
        return _EXP_OPS
    import concourse.dve_ops as dve_ops
    from concourse.dve_ops import DveOp, DveOpSpec, get_dve_sub_opcode
    from concourse.dve_spec import Spec, Src0, C0, C1, C2, One, sq, lower

    existing = {op.name: op for op in dve_ops.OPS}
    if "EXPA_ANT" in existing and "SQ4_ANT" in existing:
        _EXP_OPS = (existing["EXPA_ANT"], existing["SQ4_ANT"])
        return _EXP_OPS

    def _ref_a(in0, in1, c0, c1, c2):
        x = in0.astype(np.float32)
        q = ((x * np.float32(c2) + np.float32(c1)) * x + np.float32(c0)) * x + np.float32(1.0)
        q = q * q
        return q * q

    def _ref_sq4(in0, in1, c0, c1, c2):
        x = in0.astype(np.float32)
        for _ in range(4):
            x = x * x
        return x

    opa = DveOp(
        "EXPA_ANT",
        Spec(body=sq(sq(((Src0 * C2 + C1) * Src0 + C0) * Src0 + One)), reference=_ref_a),
        subdim=False,
        uops_sha={},
    )
    opb = DveOp(
        "SQ4_ANT",
        Spec(body=sq(sq(sq(sq(Src0)))), reference=_ref_sq4),
        subdim=False,
        uops_sha={},
    )
    for op in (opa, opb):
        dve_ops.OPS.append(op)
        dve_ops._SUB_OPCODE_FOR_NAME[op.name] = (
            max(dve_ops._SUB_OPCODE_FOR_NAME.values()) + 1
        )
        dve_ops.CUSTOM_DVE_SPECS[op.name] = op.spec
        for ver in ("v3", "v4"):
            try:
                spec_c = DveOpSpec(
                    name=op.name,
                    opcode=get_dve_sub_opcode(op.name),
                    uops=lower(op.spec, ver=ver),
                    rd1_en=False,
                )
                op.uops_sha[ver] = spec_c.sha(ver)
            except Exception:
                pass
    _EXP_OPS = (opa, opb)
    return _EXP_OPS


EXPA_CONSTS = {
    "s0": 1.0 / 512.0,
    "s1": 1.0 / (2.0 * 512.0**2),
    "imm2": 1.0 / (6.0 * 512.0**3),
}

# per-group kts whose exp pair goes to the DVE (2-op wide chain); the rest
# go to ScalarE as one wide ACTIVATE.  Alternating 5/4 DVE pairs balances
# ACT (~1147ns/pair) and DVE (~2384ns/pair + ~2556ns recip+muls) at
# ~13.2us/group each.
_DVE_KTS_A = frozenset({0, 3, 6, 9, 12})
_DVE_KTS_B = frozenset({0, 3, 6, 9})

_BUILT = None  # cached compiled Bass program


def _build_program():
    global _BUILT
    if _BUILT is not None:
        return _BUILT

    import concourse.bass as bass
    import concourse.mybir as mybir
    from concourse import bacc
    from concourse.tile import TileContext

    EXPA, SQ4 = _register_exp_ops()

    F32 = mybir.dt.float32
    BF16 = mybir.dt.bfloat16
    AF = mybir.ActivationFunctionType

    nc = bacc.Bacc("TRN2", target_bir_lowering=False, debug=False, num_devices=NCORES)

    xq = nc.dram_tensor("xq", [E, S], BF16, kind="ExternalInput")
    xk = nc.dram_tensor("xk", [E, S], BF16, kind="ExternalInput")
    xv = nc.dram_tensor("xv", [E, S], BF16, kind="ExternalInput")
    wq = nc.dram_tensor("wq", [E, F], BF16, kind="ExternalInput")
    wk = nc.dram_tensor("wk", [E, F], BF16, kind="ExternalInput")
    wv = nc.dram_tensor("wv", [E, F], BF16, kind="ExternalInput")
    wo = nc.dram_tensor("wo", [F, E], BF16, kind="ExternalInput")
    bq = nc.dram_tensor("bq", [F], F32, kind="ExternalInput")
    bk = nc.dram_tensor("bk", [F], F32, kind="ExternalInput")
    out_d = nc.dram_tensor("out", [E, S], BF16, kind="ExternalOutput")

    with TileContext(nc) as tc:
        with (
            tc.tile_pool(name="persist", bufs=1) as persist,
            tc.tile_pool(name="xp", bufs=2) as xp,
            tc.tile_pool(name="ptp", bufs=4) as ptp,
            tc.tile_pool(name="smp", bufs=2) as smp,
            tc.tile_pool(name="ost", bufs=4) as ostp,
        ):
            QT = persist.tile([128, 4, S], BF16)
            KT = persist.tile([128, 4, S], BF16)
            Vn = persist.tile([128, 16, 8, 65], BF16)
            CX = persist.tile([128, 4, S], BF16)

            # ---------------- P0: ACT exp-table preload ----------
            junk = persist.tile([1, 8], BF16)
            nc.vector.memset(junk, 0.0)
            jexp = persist.tile([1, 8], F32)
            nc.scalar.activation(out=jexp, in_=junk, func=AF.Exp, scale=SCALE)

            # ---------------- P1: projections ----------------
            with (
                tc.tile_pool(name="wp1", bufs=1) as wp1,
                tc.tile_pool(name="mm1", bufs=3, space="PSUM") as mm1,
            ):
                # first DMAs on the queue: what the first matmul needs
                xv_r = xv.rearrange("(ec p) s -> p ec s", p=128)
                xch_next = xp.tile([128, 8, 512], BF16, tag="x", name="xch0")
                wv_sb = wp1.tile([128, 8, F], BF16, tag="wv")
                wv_r = wv.rearrange("(ec p) f -> p ec f", p=128)
                # split the first loads so the first matmul (needs ec=0 only)
                # starts as early as possible
                nc.sync.dma_start(out=xch_next[:, 0:2, :], in_=xv_r[:, 0:2, 0:512])
                nc.sync.dma_start(out=wv_sb[:, 0:2, :], in_=wv_r[:, 0:2, :])
                nc.sync.dma_start(out=xch_next[:, 2:8, :], in_=xv_r[:, 2:8, 0:512])
                nc.sync.dma_start(out=wv_sb[:, 2:8, :], in_=wv_r[:, 2:8, :])
                biases = persist.tile([128, 2, 4], F32)
                for ti, bt in enumerate((bq, bk)):
                    nc.sync.dma_start(
                        out=biases[:, ti, :],
                        in_=bt.rearrange("(ft p) -> p ft", p=128),
                    )
                # ones column for the rowsum trick (V stationary col 64)
                onec = persist.tile([128, 16, 8, 1], F32)
                nc.vector.memset(onec, 1.0)
                nc.vector.tensor_copy(out=Vn[:, :, :, 64:65], in_=onec)

                wq_sb = wp1.tile([128, 8, F], BF16, tag="wq")
                wk_sb = wp1.tile([128, 8, F], BF16, tag="wk")
                qk_xr = (
                    xq.rearrange("(ec p) s -> p ec s", p=128),
                    xk.rearrange("(ec p) s -> p ec s", p=128),
                )
                qk_stream = [(0, wq_sb, QT)] * 4 + [(1, wk_sb, KT)] * 4
                qk_next = None

                # V first: produced transposed ([s, f] = ctx-stationary layout)
                for sc in range(4):
                    xch = xch_next
                    if sc < 3:
                        ssl_n = slice((sc + 1) * 512, (sc + 2) * 512)
                        xch_next = xp.tile(
                            [128, 8, 512], BF16, tag="x", name=f"xch{sc+1}"
                        )
                        nc.sync.dma_start(out=xch_next, in_=xv_r[:, :, ssl_n])
                    for st in range(4):
                        stsl = slice(st * 128, (st + 1) * 128)
                        p = mm1.tile([128, 512], F32, tag="mm")
                        for ec in range(8):
                            nc.tensor.matmul(
                                p,
                                xch[:, ec, stsl],
                                wv_sb[:, ec, :],
                                start=(ec == 0),
                                stop=(ec == 7),
                            )
                        kti = sc * 4 + st
                        nc.vector.tensor_copy(
                            out=Vn[:, kti, :, 0:64],
                            in_=p.rearrange("p (h d) -> p h d", h=8),
                        )
                    if sc == 0:
                        # issue Q/K weight loads while V computes
                        nc.sync.dma_start(
                            out=wq_sb, in_=wq.rearrange("(ec p) f -> p ec f", p=128)
                        )
                        nc.sync.dma_start(
                            out=wk_sb, in_=wk.rearrange("(ec p) f -> p ec f", p=128)
                        )
                    if sc == 2:
                        # prefetch the Q loop's first x chunk during V's tail
                        qk_next = xp.tile(
                            [128, 8, 512], BF16, tag="x", name="qkch0"
                        )
                        nc.sync.dma_start(
                            out=qk_next, in_=qk_xr[0][:, :, 0:512]
                        )

                # Q, K: W stationary, x moving; bias added on eviction
                # (ScalarE).  x chunks are prefetched one ahead (the first
                # was issued during the V loop) so chunk-boundary DMA waits
                # vanish.
                for ci, (ti, wsb, dst) in enumerate(qk_stream):
                    sc = ci % 4
                    ssl = slice(sc * 512, (sc + 1) * 512)
                    xch = qk_next
                    if ci + 1 < len(qk_stream):
                        nti = qk_stream[ci + 1][0]
                        nsc = (ci + 1) % 4
                        qk_next = xp.tile(
                            [128, 8, 512], BF16, tag="x", name=f"qkch{ci+1}"
                        )
                        nc.sync.dma_start(
                            out=qk_next,
                            in_=qk_xr[nti][:, :, nsc * 512 : (nsc + 1) * 512],
                        )
                    if True:
                        for ft in range(4):
                            fsl = slice(ft * 128, (ft + 1) * 128)
                            p = mm1.tile([128, 512], F32, tag="mm")
                            for ec in range(8):
                                nc.tensor.matmul(
                                    p,
                                    wsb[:, ec, fsl],
                                    xch[:, ec, :],
                                    start=(ec == 0),
                                    stop=(ec == 7),
                                )
                            nc.scalar.add(
                                out=dst[:, ft, ssl],
                                in_=p,
                                add=biases[:, ti, ft : ft + 1],
                            )

            # ---------------- P2: attention ----------------
            with tc.tile_pool(name="wp2", bufs=1) as wp2:
              wo_sb = wp2.tile([128, 4, E], BF16, tag="wo")
              nc.sync.dma_start(
                  out=wo_sb, in_=wo.rearrange("(fc p) e -> p fc e", p=128)
              )
              with (
                tc.tile_pool(name="scp", bufs=2, space="PSUM") as scp,
                tc.tile_pool(name="cxp", bufs=2, space="PSUM") as cxp,
              ):
                def scores(pr, qsl, kt):
                    ksl = slice(kt * 128, (kt + 1) * 128)
                    sp = scp.tile([128, 1024], F32, tag="sc", name=f"sp_{kt}")
                    nc.tensor.matmul(
                        sp[:, 0:512],
                        KT[0:64, pr, ksl], QT[0:64, pr, qsl],
                        start=True, stop=True, tile_position=(0, 0),
                    )
                    nc.tensor.matmul(
                        sp[:, 512:1024],
                        KT[64:128, pr, ksl], QT[64:128, pr, qsl],
                        start=True, stop=True, tile_position=(64, 0),
                    )
                    return sp

                def exp_pair(sp, kt, dve_kts):
                    ptt = ptp.tile(
                        [128, 1024], BF16, tag="pt", bufs=13, name=f"pt_{kt}"
                    )
                    if kt in dve_kts:
                        escr = ptp.tile(
                            [128, 1024], F32, tag="escr", bufs=2,
                            name=f"escr_{kt}",
                        )
                        nc.vector._custom_dve(
                            EXPA, out=escr, in0=sp, **EXPA_CONSTS
                        )
                        nc.vector._custom_dve(SQ4, out=ptt, in0=escr)
                    else:
                        nc.scalar.activation(
                            out=ptt, in_=sp, func=AF.Exp, scale=SCALE
                        )
                    return ptt

                norm_pending = []

                def emit_ctx(job):
                    cp, pr, qsl, kt, ptt = job
                    nc.tensor.matmul(
                        cp[:, 0:512], Vn[:, kt, 2 * pr, :], ptt[:, 0:512],
                        start=(kt == 0), stop=(kt == 15),
                    )
                    nc.tensor.matmul(
                        cp[:, 512:1024], Vn[:, kt, 2 * pr + 1, :],
                        ptt[:, 512:1024],
                        start=(kt == 0), stop=(kt == 15),
                    )
                    if kt == 15:
                        # normalize: CX[:, pr, qsl] = ctx / rowsum.
                        # DVE reads PSUM correctly only at partition base 0,
                        # so take the reciprocal of the WHOLE ctx tile (rows
                        # 0-63 are garbage, never read; DVE time is free-dim
                        # bound so this costs the same as a [1,1024] recip) —
                        # row 64 = 1/rowsums.  A tiny sbuf->sbuf DMA (idle
                        # queue) moves it to partition 0 for the broadcast.
                        # This removes the former 1.15us/group ScalarE copy.
                        rc = smp.tile([65, 1024], F32, tag="rc")
                        nc.vector.reciprocal_approx_fast(
                            out=rc, in_=cp[0:65, 0:1024]
                        )
                        inv = smp.tile([1, 1024], F32, tag="inv")
                        nc.sync.dma_start(out=inv, in_=rc[64:65, 0:1024])
                        invB = smp.tile([64, 1024], F32, tag="invB")
                        nc.gpsimd.partition_broadcast(out_ap=invB, in_ap=inv)
                        # the muls wait on the DMA-shift -> broadcast chain
                        # (~3.5us); delay their emission so DVE exp pairs slot
                        # in between instead of queueing behind them
                        norm_pending.append([4, cp, pr, qsl, invB])

                def emit_muls(job):
                    _, cp, pr, qsl, invB = job
                    nc.vector.tensor_mul(
                        CX[0:64, pr, qsl], cp[0:64, 0:512], invB[:, 0:512]
                    )
                    nc.vector.tensor_mul(
                        CX[64:128, pr, qsl], cp[0:64, 512:1024],
                        invB[:, 512:1024]
                    )

                def tick_norms():
                    for job in list(norm_pending):
                        job[0] -= 1
                        if job[0] <= 0:
                            norm_pending.remove(job)
                            emit_muls(job)

                # ctx trails scores/exp by LAG kts: by the time a ctx pair
                # issues, its exp (pt in SBUF, deep pool) finished long ago,
                # so the PE never waits on an in-flight ACT/DVE op.
                LAG = 10
                pending = []
                for qb in range(4):
                    qsl = slice(qb * 512, (qb + 1) * 512)
                    for pr in range(4):
                        g = qb * 4 + pr
                        dve_kts = _DVE_KTS_B if g % 2 == 1 else _DVE_KTS_A
                        cp = cxp.tile([65, 1024], F32, tag="cx")
                        sp_ = scores(pr, qsl, 0)
                        for kt in range(16):
                            ptt = exp_pair(sp_, kt, dve_kts)
                            if kt < 15:
                                sp_ = scores(pr, qsl, kt + 1)
                            pending.append((cp, pr, qsl, kt, ptt))
                            while len(pending) > LAG:
                                emit_ctx(pending.pop(0))
                                tick_norms()
                while pending:
                    emit_ctx(pending.pop(0))
                    tick_norms()
                while norm_pending:
                    emit_muls(norm_pending.pop(0))

              # ---------------- P3: output projection ----------------
              with tc.tile_pool(name="mmo", bufs=4, space="PSUM") as mmo:
                  for qb in range(4):
                      qsl = slice(qb * 512, (qb + 1) * 512)
                      for et in range(8):
                          esl = slice(et * 128, (et + 1) * 128)
                          p = mmo.tile([128, 512], F32, tag="mm")
                          for fc in range(4):
                              nc.tensor.matmul(
                                  p, wo_sb[:, fc, esl], CX[:, fc, qsl],
                                  start=(fc == 0), stop=(fc == 3),
                              )
                          o = ostp.tile([128, 512], BF16, tag="ost")
                          if (qb * 8 + et) % 2 == 0:
                              nc.scalar.copy(out=o, in_=p)
                          else:
                              nc.vector.tensor_copy(out=o, in_=p)
                          nc.sync.dma_start(out=out_d[esl, qsl], in_=o)

    nc.compile()
    _BUILT = nc
    return nc


def _to_bf16(x: np.ndarray):
    import ml_dtypes

    return np.ascontiguousarray(x).astype(ml_dtypes.bfloat16)


def _make_in_maps(inputs):
    query = np.asarray(inputs["query"], dtype=np.float32)
    key_ = np.asarray(inputs["key_"], dtype=np.float32)
    value = np.asarray(inputs["value"], dtype=np.float32)
    Wq = np.asarray(inputs["Wq"], dtype=np.float32)
    bq = np.asarray(inputs["bq"], dtype=np.float32)
    Wk = np.asarray(inputs["Wk"], dtype=np.float32)
    bk = np.asarray(inputs["bk"], dtype=np.float32)
    Wv = np.asarray(inputs["Wv"], dtype=np.float32)
    Wo = np.asarray(inputs["Wo"], dtype=np.float32)

    WqT = _to_bf16(Wq.T)  # [E_in, E_out]
    WkT = _to_bf16(Wk.T)
    WvT = _to_bf16(Wv.T)
    WoT = _to_bf16(Wo.T)  # [F_in, E_out]

    in_maps = []
    for c in range(NCORES):
        b = c // 2
        hh = c % 2
        fsl = slice(hh * F, (hh + 1) * F)
        in_maps.append(
            {
                "xq": _to_bf16(query[b].T),
                "xk": _to_bf16(key_[b].T),
                "xv": _to_bf16(value[b].T),
                "wq": np.ascontiguousarray(WqT[:, fsl]),
                "wk": np.ascontiguousarray(WkT[:, fsl]),
                "wv": np.ascontiguousarray(WvT[:, fsl]),
                "wo": np.ascontiguousarray(WoT[fsl, :]),
                "bq": np.ascontiguousarray(bq[fsl]),
                "bk": np.ascontiguousarray(bk[fsl]),
            }
        )
    return in_maps


def kernel(**inputs) -> np.ndarray:
    from concourse.bass_utils import run_bass_kernel_spmd

    nc = _build_program()
    in_maps = _make_in_maps(inputs)

    bv = np.asarray(inputs["bv"], dtype=np.float32)
    bo = np.asarray(inputs["bo"], dtype=np.float32)
    Wo = np.asarray(inputs["Wo"], dtype=np.float32)
    bo_prime = bo + Wo @ bv  # V-bias folded through softmax + out-proj

    res = run_bass_kernel_spmd(nc, in_maps, core_ids=list(range(NCORES)))

    out = np.empty((B, S, E), dtype=np.float32)
    for b in range(B):
        partial = res.results[2 * b]["out"].astype(np.float32) + res.results[
            2 * b + 1
        ]["out"].astype(np.float32)  # [E, S]
        out[b] = partial.T + bo_prime[None, :]
    return out
